# revision 1
# baseline (speedup 1.0000x reference)
"""Trainium2 Bass kernel for APL Kuramoto layer (B=128, N=1024, 10 steps).

Math: per step, coupling_sum[b,i] = sum_j K[i,j] sin(theta_j - theta_i)
    = cos(theta_i) * (K @ sin(theta))[i] - sin(theta_i) * (K @ cos(theta))[i]
so each step is two batched matvecs against K (symmetric) plus pointwise work.

Strategy (pure data-parallel, zero collectives — collective floors on trn2 are
~5-10us per call which would dominate 10 sequential steps):
  - shard batch 128 -> 16 rows per core; replicate K (pre-scaled by
    DT*K_global/n, bf16) to all 8 cores.
  - everything on-device lives in "T layout": [128 partitions, blocks x batch]
    where partition p of block t is oscillator t*128+p. Matmuls use K tiles as
    stationary weights (bf16 -> fast weight load) streaming sin|cos columns.
  - theta accumulates UNWRAPPED in f32; before each sin/cos the argument is
    wrapped into [-pi, pi] with the float32 magic-number round (ACT's Sin
    spline is only valid on [-pi, pi]); cos(x) = sin(wrap(x + pi/2)).
  - final arctan2-wrap and the coherence reduction happen on host (numpy),
    matching the reference's output semantics exactly.
"""
import numpy as np
from contextlib import ExitStack

import concourse.bass as bass
import concourse.tile as tile
import concourse.bacc as bacc
from concourse import mybir
from concourse.bass_utils import run_bass_kernel_spmd

import ml_dtypes

P = 128          # partitions
NT = 8           # oscillator tiles (1024 / 128)
BL = 16          # batch rows per core
NC = 8           # cores
N = NT * P       # 1024 oscillators
B = NC * BL      # 128 batch
STEPS = 10
DT = 0.1
SCW = 2 * BL     # sin|cos block width (32)
HALF_T = NT // 2 # i-tiles per half

F32 = mybir.dt.float32
BF16 = mybir.dt.bfloat16

TWO_PI = float(2.0 * np.pi)
INV_2PI = float(np.float32(1.0 / (2.0 * np.pi)))
HALF_PI = float(np.pi / 2)
MAGIC = float(np.float32(1.5 * 2 ** 23))  # f32 RNE round-to-int magic


def _emit_wrap_sincos(nc, wk, theta_ap, sc_out_ap, nblk):
    """From theta (T layout [128, nblk*BL], unwrapped), write sin|cos blocks
    ([BL sin | BL cos] per block) into sc_out_ap [128, nblk*SCW] (bf16).

    thw = theta - 2pi*round(theta/2pi) in [-pi, pi];  sin <- Sin(thw)
    thc = thw + pi/2 - 2pi*(thw > pi/2) in [-pi, pi]; cos <- Sin(thc)
    """
    FD = nblk * BL
    m2 = wk.tile([P, FD], F32, tag="m2")
    nc.vector.tensor_scalar(m2[:], theta_ap, INV_2PI, MAGIC,
                            mybir.AluOpType.mult, mybir.AluOpType.add)
    m3 = wk.tile([P, FD], F32, tag="m3")
    nc.vector.tensor_scalar(m3[:], m2[:], MAGIC, TWO_PI,
                            mybir.AluOpType.subtract, mybir.AluOpType.mult)
    # thwc holds [BL thw | BL thc] per block, matching sc layout
    thwc = wk.tile([P, nblk, SCW], F32, tag="thwc")
    thw = thwc[:, :, 0:BL]
    nc.vector.tensor_sub(thw, theta_ap.rearrange("p (t b) -> p t b", t=nblk), m3[:].rearrange("p (t b) -> p t b", t=nblk))
    # g = (thw > pi/2) - 0.25 ;  thc = g*(-2pi) + thw  (= thw + pi/2 - 2pi*[thw>pi/2])
    g = wk.tile([P, nblk, BL], F32, tag="g")
    nc.vector.tensor_scalar(g[:], thw, HALF_PI, 0.25,
                            mybir.AluOpType.is_gt, mybir.AluOpType.subtract)
    nc.vector.scalar_tensor_tensor(thwc[:, :, BL:SCW], g[:], -TWO_PI, thw,
                                   mybir.AluOpType.mult, mybir.AluOpType.add)
    # one ACT pass: sin over both halves
    zb = _emit_wrap_sincos._zero_bias
    nc.scalar.activation(out=sc_out_ap, in_=thwc[:],
                         func=mybir.ActivationFunctionType.Sin, bias=zb)


def build_nc():
    nc = bacc.Bacc("TRN2", target_bir_lowering=False, debug=False, num_devices=NC)
    ks_d = nc.declare_dram_parameter("ks", [P, NT * N], BF16, isOutput=False)
    th_d = nc.declare_dram_parameter("theta0", [P, NT * BL], F32, isOutput=False)
    om_d = nc.declare_dram_parameter("omega_b", [P, NT * BL], F32, isOutput=False)
    out_d = nc.declare_dram_parameter("out", [P, NT * BL], F32, isOutput=True)

    with tile.TileContext(nc) as tc, ExitStack() as ctx:
        singles = ctx.enter_context(tc.tile_pool(name="singles", bufs=1))
        scp = ctx.enter_context(tc.tile_pool(name="scp", bufs=2))
        wk = ctx.enter_context(tc.tile_pool(name="wk", bufs=3))
        psum = ctx.enter_context(tc.tile_pool(name="psum", bufs=4, space="PSUM"))

        zero_b = singles.tile([P, 1], F32)
        nc.vector.memset(zero_b[:], 0.0)
        _emit_wrap_sincos._zero_bias = zero_b[:]

        theta = singles.tile([P, NT * BL], F32)
        nc.sync.dma_start(out=theta[:], in_=th_d.ap())
        omega_b = singles.tile([P, NT * BL], F32)
        nc.sync.dma_start(out=omega_b[:], in_=om_d.ap())
        ks = singles.tile([P, NT * N], BF16)
        for j in range(NT):
            nc.sync.dma_start(out=ks[:, j * N:(j + 1) * N],
                              in_=ks_d.ap()[:, j * N:(j + 1) * N])

        # prologue: sc0 = sin|cos(theta0)
        sc = scp.tile([P, NT * SCW], BF16, tag="sc")
        _emit_wrap_sincos(nc, wk, theta[:], sc[:].rearrange("p (t w) -> p t w", t=NT), NT)

        for s in range(STEPS):
            # theta-plus-omega, full width; overlaps with the matmuls
            thp = wk.tile([P, NT * BL], F32, tag="thp")
            nc.vector.tensor_add(thp[:], theta[:], omega_b[:])

            sc_next = None
            if s < STEPS - 1:
                sc_next = scp.tile([P, NT * SCW], BF16, tag="sc")

            for h in range(2):
                ps = psum.tile([P, HALF_T * SCW], F32)
                for j in range(NT):
                    rhs = sc[:, j * SCW:(j + 1) * SCW]
                    for il in range(HALF_T):
                        i = h * HALF_T + il
                        nc.tensor.matmul(
                            out=ps[:, il * SCW:(il + 1) * SCW],
                            lhsT=ks[:, j * N + i * P: j * N + (i + 1) * P],
                            rhs=rhs,
                            start=(j == 0), stop=(j == NT - 1),
                        )
                # combine: theta_h = (thp_h + cos*S) - sin*C
                psv = ps[:].rearrange("p (t w) -> p t w", t=HALF_T)
                scv = sc[:, h * HALF_T * SCW:(h + 1) * HALF_T * SCW] \
                    .rearrange("p (t w) -> p t w", t=HALF_T)
                FDh = HALF_T * BL
                t1 = wk.tile([P, HALF_T, BL], F32, tag="t1")
                nc.vector.tensor_mul(t1[:], scv[:, :, BL:SCW], psv[:, :, 0:BL])
                t2 = wk.tile([P, HALF_T, BL], F32, tag="t2")
                nc.vector.tensor_mul(t2[:], scv[:, :, 0:BL], psv[:, :, BL:SCW])
                a = wk.tile([P, HALF_T, BL], F32, tag="a")
                nc.vector.tensor_add(
                    a[:], thp[:, h * FDh:(h + 1) * FDh].rearrange("p (t b) -> p t b", t=HALF_T), t1[:])
                th_h = theta[:, h * FDh:(h + 1) * FDh]
                nc.vector.tensor_sub(th_h.rearrange("p (t b) -> p t b", t=HALF_T), a[:], t2[:])
                if sc_next is not None:
                    _emit_wrap_sincos(
                        nc, wk, th_h,
                        sc_next[:, h * HALF_T * SCW:(h + 1) * HALF_T * SCW]
                        .rearrange("p (t w) -> p t w", t=HALF_T),
                        HALF_T)
            sc = sc_next

        nc.sync.dma_start(out=out_d.ap(), in_=theta[:])

    nc.compile()
    return nc


_NC_CACHE = None


def _get_nc():
    global _NC_CACHE
    if _NC_CACHE is None:
        _NC_CACHE = build_nc()
    return _NC_CACHE


def kernel(theta_init, K, omega, K_global, _want_timing=False):
    theta_init = np.asarray(theta_init, np.float32)
    K = np.asarray(K, np.float32)
    omega = np.asarray(omega, np.float32)
    kg = float(np.asarray(K_global, np.float32))

    # host-side constant folding + layouts
    ks = (K * np.float32(DT * kg / N)).astype(np.float32)
    # ks_t[p, j*N + n] = ks[j*128 + p, n]  (row-tile major)
    ks_t = np.ascontiguousarray(
        ks.reshape(NT, P, N).transpose(1, 0, 2).reshape(P, NT * N)
    ).astype(ml_dtypes.bfloat16)
    om_b = np.repeat((DT * omega).astype(np.float32).reshape(NT, P).T[:, :, None],
                     BL, axis=2).reshape(P, NT * BL)
    om_b = np.ascontiguousarray(om_b, dtype=np.float32)

    in_maps = []
    for c in range(NC):
        shard = theta_init[c * BL:(c + 1) * BL]                    # [16, 1024]
        th_t = np.ascontiguousarray(
            shard.reshape(BL, NT, P).transpose(2, 1, 0).reshape(P, NT * BL),
            dtype=np.float32)
        in_maps.append({"ks": ks_t, "theta0": th_t, "omega_b": om_b})

    nc = _get_nc()
    res = run_bass_kernel_spmd(nc, in_maps, core_ids=list(range(NC)),
                               trace=bool(_want_timing))

    theta_out = np.empty((B, N), np.float32)
    for c in range(NC):
        o = np.asarray(res.results[c]["out"], np.float32)          # [128, 128]
        theta_out[c * BL:(c + 1) * BL] = (
            o.reshape(P, NT, BL).transpose(2, 1, 0).reshape(BL, N))

    theta_w = np.arctan2(np.sin(theta_out), np.cos(theta_out)).astype(np.float32)
    coh = np.sqrt(np.cos(theta_w).mean(-1) ** 2 + np.sin(theta_w).mean(-1) ** 2)
    out = (theta_w, coh.astype(np.float32))
    if _want_timing:
        return out, res
    return out
